# revision 72
# baseline (speedup 1.0000x reference)
"""Windowed 3D attention with dynamic position bias — Trainium2, 8 NeuronCores.

Sharding: data-parallel over the window dim B_=64 (8 windows per core).

Per-core pipeline (per window):
  x arrives from the host already transposed to feature-major bf16
  ([p, c_chunk, tok_chunk, tok]) -- one DMA per window, half the bytes of
  the f32 original and no device-side transpose machinery at all.
  Q^T,K^T feature-major + V token-major (PE matmuls, PSUM->SBUF copies)
  per head: S^T = K^T.T Q^T (PE) -> exp on ACT -> multiply by exp(B^T) (DVE)
  -> PV with P^T chunks as the *stationary* matmul operand and V+ones as the
  33-wide moving operand (4x fewer PE cycles than the V-stationary form),
  giving token-major output with the softmax denominator as column 32 ->
  per-partition reciprocal + broadcast normalize (DVE) -> bf16 token-major O
  --DMA-transpose--> feature-major -> proj -> bf16 store (host casts to f32
  and adds proj_b only if nonzero, which it never is for this problem).

Scheduling: x loads prefetched two windows ahead; proj/store deferred one
window so the O-transpose DMA latency hides under the next window's heads;
ebt (exp of the bias table) loads interleaved with early x loads by
transfer deadline; dummy warm-up matmuls pre-ramp the PE clock p-state.

Host precomputes the tiny DynamicPosBias MLP table (L=3375 rows, ~6 MFLOP)
plus layout/dtype prep for x and the weights.
"""

import os
import numpy as np
import ml_dtypes

DIM = 384
HEADS = 12
B_WIN = 64
N_TOK = 512
NCORES = 8
WPC = B_WIN // NCORES     # windows per core
D_HEAD = DIM // HEADS     # 32
SCALE = D_HEAD ** -0.5
VA = D_HEAD + 1           # 33: head slot width in V-augmented (ones column)

LAST_RESULT = None
_CACHE = {}


def _pos_mlp_table(pos_proj_w, pos_proj_b, ln1_g, ln1_b, pos1_w, pos1_b,
                   ln2_g, ln2_b, pos2_w, pos2_b, ln3_g, ln3_b, pos3_w, pos3_b):
    """Host replica of the reference DynamicPosBias MLP. Returns (L, HEADS)."""
    H = W = D = 8
    rh = np.arange(1 - H, H)
    biases = np.stack(np.meshgrid(rh, rh, rh, indexing="ij"))
    biases = biases.reshape(3, -1).T.astype(np.float32)

    def ln(x, g, b):
        m = x.mean(-1, keepdims=True)
        v = x.var(-1, keepdims=True)
        return (x - m) / np.sqrt(v + 1e-5) * g + b

    p = biases @ pos_proj_w + pos_proj_b
    p = np.maximum(ln(p, ln1_g, ln1_b), 0.0) @ pos1_w + pos1_b
    p = np.maximum(ln(p, ln2_g, ln2_b), 0.0) @ pos2_w + pos2_b
    p = np.maximum(ln(p, ln3_g, ln3_b), 0.0) @ pos3_w + pos3_b
    return p.astype(np.float32)


def _rpi():
    H = W = D = 8
    coords = np.stack(np.meshgrid(np.arange(H), np.arange(W), np.arange(D),
                                  indexing="ij")).reshape(3, -1)
    rel = (coords[:, :, None] - coords[:, None, :]).transpose(1, 2, 0)
    rel = rel + np.array([H - 1, W - 1, D - 1])
    rel = rel * np.array([(2 * W - 1) * (2 * D - 1), 2 * D - 1, 1])
    return rel.sum(-1)  # (N, N) int


def _build():
    import concourse.mybir as mybir
    import concourse.tile as tile

    f32 = mybir.dt.float32
    bf16 = mybir.dt.bfloat16
    Exp = mybir.ActivationFunctionType.Exp
    Mult = mybir.AluOpType.mult

    from concourse import bacc
    nc = bacc.Bacc(None)
    x_ext = nc.declare_dram_parameter("x", [WPC, 128, 3, 4, 128], bf16, isOutput=False)
    ebt_ext = nc.declare_dram_parameter("ebt", [HEADS, N_TOK, N_TOK], bf16, isOutput=False)
    qkvw_ext = nc.declare_dram_parameter("qkvw", [DIM, 3 * DIM], bf16, isOutput=False)
    projw_ext = nc.declare_dram_parameter("projw", [DIM, DIM], bf16, isOutput=False)
    identb_ext = nc.declare_dram_parameter("identb", [128, 128], bf16, isOutput=False)
    id8_ext = nc.declare_dram_parameter("id8", [64, 2, 128], mybir.dt.float8e4, isOutput=False)
    b8_ext = nc.declare_dram_parameter("b8", [4, 64, 2, N_TOK], mybir.dt.float8e4, isOutput=False)
    out_ext = nc.declare_dram_parameter("out", [WPC, N_TOK, DIM], bf16, isOutput=True)

    with tile.TileContext(nc) as tc:
        with (
            tc.tile_pool(name="const", bufs=1) as cpool,
            tc.tile_pool(name="xt", bufs=4) as xtp,
            tc.tile_pool(name="qk", bufs=30) as qkp,
            tc.tile_pool(name="va", bufs=16) as vap,
            tc.tile_pool(name="pe", bufs=6) as pep,
            tc.tile_pool(name="rd", bufs=8) as rdp,
            tc.tile_pool(name="otk", bufs=3) as otkp,
            tc.tile_pool(name="oft", bufs=3) as oftp,
            tc.tile_pool(name="y", bufs=3) as yp,
            tc.tile_pool(name="psS", bufs=2, space="PSUM") as psS,
            tc.tile_pool(name="psQ", bufs=2, space="PSUM") as psQ,
            tc.tile_pool(name="psP", bufs=1, space="PSUM") as psP,
            tc.tile_pool(name="psO", bufs=1, space="PSUM") as psO,
        ):
            # ---- PE p-state warm-up: dummy matmuls to start the ramp ----
            wdum = cpool.tile([128, N_TOK], bf16, tag="wdum")
            nc.vector.memset(wdum[:], 0.0)
            for _ in range(4):
                wps = psS.tile([128, 2, N_TOK], f32, tag="st", name="warm")
                nc.tensor.matmul(wps[:, 0, :], wdum[:, 0:128], wdum[:],
                                 start=True, stop=True)

            # ---- x arrives pre-transposed bf16 from the host: one DMA each ----
            def xt_load(b):
                xt = xtp.tile([128, 3, 4, 128], bf16, tag="xt", name=f"xt{b}")
                nc.sync.dma_start(xt[:], x_ext[b])
                return xt

            def ebt_load(h):
                nc.sync.dma_start(ebt[:, h],
                                  ebt_ext[h].rearrange("(mm p) n -> p mm n", p=128))

            ebt = cpool.tile([128, HEADS, 4, N_TOK], bf16, tag="ebt")
            xt_queue = [xt_load(0)]
            emitted = [True, True, True] + [False] * (WPC - 3)
            pending_proj = None

            # ---- constants + ebt/x loads interleaved by transfer deadline ----
            # qkvw: head-0's Q/K column blocks first so window 0 starts early
            qkvw = cpool.tile([128, 3, 3 * DIM], bf16, tag="qkvw")
            qkvw_r = qkvw_ext.rearrange("(c p) n -> p c n", p=128)
            nc.sync.dma_start(qkvw[:, :, 0:128], qkvw_r[:, :, 0:128])
            nc.sync.dma_start(qkvw[:, :, 384:512], qkvw_r[:, :, 384:512])
            nc.sync.dma_start(qkvw[:, :, 128:384], qkvw_r[:, :, 128:384])
            nc.sync.dma_start(qkvw[:, :, 512:1152], qkvw_r[:, :, 512:1152])
            projw = cpool.tile([128, 3, DIM], bf16, tag="projw")
            nc.sync.dma_start(projw[:], projw_ext.rearrange("(c p) n -> p c n", p=128))
            identb = cpool.tile([128, 128], bf16, tag="identb")
            nc.sync.dma_start(identb[:], identb_ext[:])
            for h in (0, 1, 2):
                ebt_load(h)
            xt_queue.append(xt_load(1))
            for h in (3, 4, 5):
                ebt_load(h)
            xt_queue.append(xt_load(2))
            for h in range(6, HEADS):
                ebt_load(h)
            id8 = cpool.tile([64, 2, 128], mybir.dt.float8e4, tag="id8")
            nc.sync.dma_start(id8[:], id8_ext[:])
            b8 = cpool.tile([64, 2, 4, 2, N_TOK], mybir.dt.float8e4, tag="b8")
            nc.sync.dma_start(b8[:, 0], b8_ext.rearrange("m p t n -> p m t n"))
            # V tiles: 12 static buffers, ones column written once
            va_all = [vap.tile([128, HEADS, VA], bf16, tag="va", name=f"vab{i}")
                      for i in range(16)]
            for i in range(16):
                nc.gpsimd.memset(va_all[i][:, :, D_HEAD], 1.0)

            def emit_qkv(b):
                xt = xt_queue.pop(0)
                # ---- Q^T, K^T feature-major (6 tiles of (128, 512)) ----
                qk = [qkp.tile([128, N_TOK], bf16, tag="qk", name=f"qk{b}_{t}")
                      for t in range(6)]
                for t in (0, 3, 1, 4, 2, 5):  # head 0 needs tiles 0,3 first
                    ps = psQ.tile([128, N_TOK], f32, tag="psq", name="psqk")
                    for c in range(3):
                        nc.tensor.matmul(ps[:],
                                         qkvw[:, c, 128 * t:128 * (t + 1)],
                                         xt[:, c], start=(c == 0), stop=(c == 2))
                    nc.vector.tensor_copy(qk[t][:], ps[:])

                # ---- V token-major (ones columns persist in va_all) ----
                va = [va_all[(4 * b + j) % 16] for j in range(4)]
                for j in range(4):
                    ps = psQ.tile([128, DIM], f32, tag="psq", name="psv")
                    for c in range(3):
                        nc.tensor.matmul(ps[:], xt[:, c, j, :],
                                         qkvw[:, c, 2 * DIM:3 * DIM],
                                         start=(c == 0), stop=(c == 2))
                    nc.vector.tensor_copy(va[j][:, :, 0:D_HEAD],
                                          ps.rearrange("p (h d) -> p h d",
                                                       d=D_HEAD))
                return qk, va

            for b in range(WPC):
                qk, va = emit_qkv(b)

                # token-major normalized output [p, nchunk, head, d]
                otk = otkp.tile([128, 4, HEADS, D_HEAD], bf16, tag="otk", name=f"otk{b}")

                def emit_norm(po, g3):
                    # normalize 3 heads: reciprocal of denom column, bcast mult
                    rd = rdp.tile([128, 3, 4], f32, tag="rd", name=f"rd{b}_{g3}")
                    nc.vector.reciprocal(rd[:], po[:, :, :, D_HEAD])
                    nc.vector.tensor_tensor(
                        otk[:, :, 3 * g3:3 * (g3 + 1), :],
                        po[:, :, :, 0:D_HEAD].rearrange("p h j d -> p j h d"),
                        rd.rearrange("p h j -> p j h").unsqueeze(3)
                          .to_broadcast([128, 4, 3, D_HEAD]),
                        op=Mult)

                po = None
                for h in range(HEADS):
                    if h == 2 and pending_proj is not None:
                        pending_proj()
                        pending_proj = None
                    if h % 3 == 0:
                        po_prev, po = po, psO.tile([128, 3, 4, VA], f32,
                                                   tag="po", name=f"po{b}_{h // 3}")
                    tq, j4 = h // 4, h % 4
                    qt, kt = qk[tq], qk[3 + tq]
                    pe = pep.tile([128, 4, N_TOK], bf16, tag="pe", name=f"pe{b}_{h}")
                    tailh = b == WPC - 1 and h == HEADS - 1
                    for half in range(2):
                        st = psS.tile([128, 2, N_TOK], f32, tag="st", name="st")
                        for mm in range(2):
                            m = 2 * half + mm
                            nc.tensor.matmul(
                                st[:, mm, :],
                                kt[32 * j4:32 * (j4 + 1), 128 * m:128 * (m + 1)],
                                qt[32 * j4:32 * (j4 + 1), :],
                                start=True, stop=(not tailh),
                                tile_position=(32 * j4, 0))
                            if tailh:
                                # bias += B^T/scale via fp8 DoubleRow identity
                                # matmul: the exp output needs no DVE multiply,
                                # shortening the final window's drain chain
                                nc.tensor.matmul(
                                    st[:, mm, :], id8[:], b8[:, 0, m],
                                    start=False, stop=True,
                                    perf_mode=mybir.MatmulPerfMode.DoubleRow)
                        nc.scalar.activation(pe[:, 2 * half:2 * (half + 1), :],
                                             st[:], Exp, scale=float(SCALE))
                    if not tailh:
                        pef = pe.rearrange("p a n -> p (a n)")
                        ebth = ebt[:, h].rearrange("p a n -> p (a n)")
                        nc.vector.tensor_mul(pef, pef, ebth)
                    if h % 3 == 0 and h > 0:
                        emit_norm(po_prev, h // 3 - 1)  # overlaps this head's PV
                    # PV: P^T chunks stationary, V+ones moving (33 wide)
                    for j in range(4):
                        for k in range(4):
                            nc.tensor.matmul(po[:, h % 3, j, :],
                                             pe[:, k, 128 * j:128 * (j + 1)],
                                             va[k][:, h, :],
                                             start=(k == 0), stop=(k == 3))
                emit_norm(po, 3)

                # ---- O^T via DMA transpose; proj/store deferred one window ----
                oft = oftp.tile([128, 3, 4, 128], bf16, tag="oft", name=f"oft{b}")
                if b == WPC - 1:
                    # tail: PE transposes avoid the DMA roundtrip latency
                    for c in range(3):
                        ps = psQ.tile([128, N_TOK], f32, tag="psq", name="psot")
                        for j in range(4):
                            nc.tensor.transpose(
                                ps.bitcast(bf16)[:, 0:N_TOK][:, 128 * j:128 * (j + 1)],
                                otk[:, j, :, :].rearrange(
                                    "p h d -> p (h d)")[:, 128 * c:128 * (c + 1)],
                                identb[:])
                        nc.vector.tensor_copy(
                            oft[:, c],
                            ps.bitcast(bf16)[:, 0:N_TOK].rearrange(
                                "p (j t) -> p j t", t=128))
                else:
                    for j in range(4):
                        nc.sync.dma_start_transpose(oft[:, :, j, :], otk[:, j])

                def make_proj(b, oft):
                    last = b == WPC - 1
                    def emit():
                        ysb = yp.tile([128, 4, DIM], bf16, tag="y", name=f"y{b}")
                        outr = out_ext[b].rearrange("(c p) f -> p c f", p=128)
                        for j in range(4):
                            # tail: psQ is idle by then; 2 bufs kill the
                            # per-j serialization, per-j stores cut latency
                            pool = psQ if last else psP
                            py = pool.tile([128, DIM], f32, tag="psq" if last
                                           else "psp", name="py")
                            for c in range(3):
                                nc.tensor.matmul(py[:], oft[:, c, j, :],
                                                 projw[:, c, :],
                                                 start=(c == 0), stop=(c == 2))
                            nc.vector.tensor_copy(ysb[:, j], py[:])
                            if last:
                                nc.sync.dma_start(outr[:, j], ysb[:, j])
                        if not last:
                            nc.sync.dma_start(outr, ysb[:])
                    return emit
                pending_proj = make_proj(b, oft)

                # prefetch x up to two windows ahead (after OT on the SP queue)
                for nb in (b + 1, b + 2):
                    if nb < WPC and not emitted[nb]:
                        xt_queue.append(xt_load(nb))
                        emitted[nb] = True
            pending_proj()
    nc.compile()
    return nc


def kernel(x, H, W, D, mask, qkv_w, qkv_b, proj_w, proj_b,
           pos_proj_w, pos_proj_b, ln1_g, ln1_b, pos1_w, pos1_b,
           ln2_g, ln2_b, pos2_w, pos2_b, ln3_g, ln3_b, pos3_w, pos3_b):
    global LAST_RESULT
    from concourse.bass_utils import run_bass_kernel_spmd

    x = np.asarray(x, np.float32)
    mask = np.asarray(mask, np.float32)
    qkv_w = np.asarray(qkv_w, np.float32)
    qkv_b = np.asarray(qkv_b, np.float32)
    proj_w = np.asarray(proj_w, np.float32)
    proj_b = np.asarray(proj_b, np.float32)

    pos = _pos_mlp_table(
        np.asarray(pos_proj_w, np.float32), np.asarray(pos_proj_b, np.float32),
        np.asarray(ln1_g, np.float32), np.asarray(ln1_b, np.float32),
        np.asarray(pos1_w, np.float32), np.asarray(pos1_b, np.float32),
        np.asarray(ln2_g, np.float32), np.asarray(ln2_b, np.float32),
        np.asarray(pos2_w, np.float32), np.asarray(pos2_b, np.float32),
        np.asarray(ln3_g, np.float32), np.asarray(ln3_b, np.float32),
        np.asarray(pos3_w, np.float32), np.asarray(pos3_b, np.float32))
    rel_bias = pos[_rpi()]                    # (N, N, HEADS), B[n, m, h]
    bt = rel_bias.transpose(2, 1, 0)          # (HEADS, m, n) = B^T per head

    if np.any(mask) or np.any(qkv_b):
        # General fallback (never taken for this problem's inputs: both zero).
        return _numpy_reference(x, mask, qkv_w, qkv_b, proj_w, proj_b, rel_bias)

    ebt = np.exp(bt).astype(ml_dtypes.bfloat16)
    # fp8 folded bias/scale for the tail head (11): b8[mc, p, t, n]
    btt = bt[HEADS - 1] / SCALE
    b8 = np.zeros((4, 64, 2, N_TOK), np.float32)
    for mc in range(4):
        for t in range(2):
            b8[mc, :, t, :] = btt[128 * mc + 64 * t:128 * mc + 64 * (t + 1), :]
    b8 = b8.astype(ml_dtypes.float8_e4m3)
    id8 = np.zeros((64, 2, 128), np.float32)
    id8[:, 0, :] = np.eye(128)[0:64]
    id8[:, 1, :] = np.eye(128)[64:128]
    id8 = id8.astype(ml_dtypes.float8_e4m3)
    qkvw_bf = qkv_w.astype(ml_dtypes.bfloat16)
    projw_bf = proj_w.astype(ml_dtypes.bfloat16)
    identb = np.eye(128, dtype=np.float32).astype(ml_dtypes.bfloat16)
    # pre-transposed feature-major bf16 x: [b, p(c_in%128), c_chunk, j(tok//128), t]
    xth = np.ascontiguousarray(np.transpose(
        x.astype(ml_dtypes.bfloat16).reshape(B_WIN, 4, 128, 3, 128),
        (0, 4, 3, 1, 2)))

    if "nc" not in _CACHE:
        _CACHE["nc"] = _build()
    nc = _CACHE["nc"]

    in_maps = []
    for c in range(NCORES):
        in_maps.append({
            "x": xth[c * WPC:(c + 1) * WPC],
            "ebt": ebt, "qkvw": qkvw_bf, "projw": projw_bf,
            "identb": identb, "id8": id8, "b8": b8,
        })
    trace = bool(os.environ.get("KBENCH_TRACE"))
    res = run_bass_kernel_spmd(nc, in_maps, list(range(NCORES)), trace=trace)
    LAST_RESULT = res
    out = np.concatenate([np.asarray(res.results[c]["out"]).astype(np.float32)
                          for c in range(NCORES)], axis=0)
    if np.any(proj_b):
        out = out + proj_b
    return out


def _numpy_reference(x, mask, qkv_w, qkv_b, proj_w, proj_b, rel_bias):
    B_, N, C = x.shape
    h, d = HEADS, D_HEAD
    qkv = (x @ qkv_w + qkv_b).reshape(B_, N, 3, h, d).transpose(2, 0, 3, 1, 4)
    q, k, v = qkv[0] * (d ** -0.5), qkv[1], qkv[2]
    attn = np.einsum("bhnd,bhmd->bhnm", q, k) + rel_bias.transpose(2, 0, 1)[None]
    nG = mask.shape[0]
    attn = (attn.reshape(B_ // nG, nG, h, N, N) + mask[None, :, None]).reshape(B_, h, N, N)
    attn = attn - attn.max(-1, keepdims=True)
    e = np.exp(attn)
    p = e / e.sum(-1, keepdims=True)
    out = np.einsum("bhnm,bhmd->bhnd", p, v).transpose(0, 2, 1, 3).reshape(B_, N, C)
    return (out @ proj_w + proj_b).astype(np.float32)
